# revision 4
# baseline (speedup 1.0000x reference)
"""Trainium2 Bass kernel for nn_ATIN_op_10926396801590 (topk_masking).

Computes idx = top_k(sigmoid(MLP(conv(x))), k=1023).indices, bit-exactly
matching the XLA-neuron reference:
  windows -> per-var conv (K=3) -> +conv_b -> W1 (C=64->H=32) -> +b1 -> tanh
  -> W2 (H=32->1) -> +b2 -> sigmoid -> stable descending top-1023 indices.

Sharding: data-parallel over batch. 8 cores x 4 batches each. Weights
replicated (host-packed into PE-friendly block-diagonal layouts). No
cross-device communication; host stacks the 8 shards.

Bit-exactness recipe (verified on hardware against jit(reference)):
- conv matmul: windows must be the STATIONARY operand (lhsT), weights moving;
  out lands [T, C]; zero-padded block-diag packing of 2 vars is bit-safe.
- feat is evicted via ACT copy, PE-transposed to [C, T], then conv_b added on
  DVE (per-partition scalar).
- W1 matmul: W1^T stationary, feat moving; tanh fused with +b1 on ACT.
- W2 matmul: 4-var block-diag [128, 4] stationary, h moving.
- sigmoid = ACT Exp(scale=-1, bias=-b2), DVE +1, DVE reciprocal
  (matches XLA's 1/(1+exp(-x)) expansion; ACT Sigmoid table does NOT match).
- top-k: 128 rounds of DVE max8 + max_index + match_replace(-1e30); max_index
  returns successive occurrence indices for duplicates == jax.lax.top_k's
  stable tie-break.
"""
import numpy as np

B, V, L, C, K, H = 32, 64, 2048, 64, 3, 32
T = L - K + 1            # 2046
TOPK = 1023
NCORES = 8
BLOC = B // NCORES       # 4 batches per core

_cached = {}


def _patch_tile_context():
    """This container's walrus accepts only ONE sync-wait command per
    instruction. Hoist extra waits onto same-engine InstNoOps and split the
    TileContext tail drain."""
    import concourse.mybir as mybir
    from concourse.tile import TileContext
    from concourse.vector_clock import ScopedClock

    if getattr(TileContext, "_single_wait_patched", False):
        return

    engine_ok = {
        mybir.EngineType.Activation,
        mybir.EngineType.DVE,
        mybir.EngineType.PE,
        mybir.EngineType.Pool,
        mybir.EngineType.SP,
    }
    counter = [0]

    orig_lower = TileContext._lower_ordered_insts

    def patched_lower(self, ordered):
        for insts in ordered.values():
            new_list = []
            for inst in insts:
                si = getattr(inst, "sync_info", None)
                waits = list(si.on_wait) if si is not None else []
                eng = getattr(inst, "engine", None)
                if len(waits) > 1 and eng in engine_ok:
                    for wt in waits[:-1]:
                        counter[0] += 1
                        nop = mybir.InstNoOp(
                            name=f"waitnop-{counter[0]}", ins=[], outs=[]
                        )
                        nop.engine = eng
                        nop.sync_info = mybir.SyncInfo(on_wait=[wt], on_update=[])
                        nop.bass_scheduled_proc = inst.bass_scheduled_proc
                        nop.bass_scheduled_tick = inst.bass_scheduled_tick
                        nop.bass_scheduled_scope = inst.bass_scheduled_scope
                        new_list.append(nop)
                    inst.sync_info = mybir.SyncInfo(
                        on_wait=[waits[-1]], on_update=list(si.on_update)
                    )
                new_list.append(inst)
            insts[:] = new_list
        return orig_lower(self, ordered)

    def patched_drain(self, tick_clock, wait_clock):
        drain_inst = self.nc.sync.drain()
        wait_clock.add_sem_waits(
            drain_inst.ins, ScopedClock({None: tick_clock.global_clock})
        )
        si = drain_inst.ins.sync_info
        waits = list(si.on_wait)
        if len(waits) > 1:
            drain_inst.ins.sync_info = mybir.SyncInfo(
                on_wait=waits[:1], on_update=list(si.on_update)
            )
            for i in range(1, len(waits)):
                extra = self.nc.sync.drain()
                extra.ins.sync_info = mybir.SyncInfo(on_wait=[waits[i]], on_update=[])
        self.nc.all_engine_barrier()
        assert self.sems is not None
        popped = self.nc._tile_sem_poison_stack.pop()
        assert popped is self._sem_poison
        self.nc.clear_and_free_semaphores(list(self.sems.allocated().values()))
        self.nc.all_engine_barrier()

    TileContext._lower_ordered_insts = patched_lower
    TileContext._drain_and_barrier = patched_drain
    TileContext._single_wait_patched = True


def _build_nc(neg_b2: float):
    import concourse.bass as bass
    import concourse.mybir as mybir
    from concourse.tile import TileContext
    from concourse.masks import make_identity

    _patch_tile_context()

    f32 = mybir.dt.float32
    nc = bass.Bass("TRN2")

    d_xs = nc.dram_tensor("xs", [BLOC, V, L], f32, kind="ExternalInput")
    d_cw = nc.dram_tensor("cw", [6, (V // 2) * 128], f32, kind="ExternalInput")
    d_cb = nc.dram_tensor("cb", [128, V // 2], f32, kind="ExternalInput")
    d_w1 = nc.dram_tensor("w1", [128, 64], f32, kind="ExternalInput")
    d_b1 = nc.dram_tensor("b1", [64, 1], f32, kind="ExternalInput")
    d_w2 = nc.dram_tensor("w2", [128, 4], f32, kind="ExternalInput")
    d_nb2 = nc.dram_tensor("nb2", [4, 1], f32, kind="ExternalInput")
    d_idx = nc.dram_tensor("idx", [2 * 128, TOPK], mybir.dt.uint32, kind="ExternalOutput")

    # position tiles per 512-chunk: widths
    CHS = [512, 512, 512, 510]
    PTW = [[128, 128, 128, 128]] * 3 + [[128, 128, 128, 126]]

    with TileContext(nc) as tc:
        with (
            tc.tile_pool(name="wts", bufs=1) as wp,
            tc.tile_pool(name="wnd", bufs=2) as wndp,
            tc.tile_pool(name="work", bufs=3) as pool,
            tc.tile_pool(name="hp", bufs=3) as hp,
            tc.tile_pool(name="blk", bufs=1) as blkp,
            tc.tile_pool(name="m8p", bufs=2) as m8p,
            tc.tile_pool(name="ps", bufs=2, space="PSUM") as psp,
        ):
            ident = wp.tile([128, 128], f32)
            make_identity(nc, ident[:])
            t_cw = wp.tile([6, (V // 2) * 128], f32)
            t_cb = wp.tile([128, V // 2], f32)
            t_w1 = wp.tile([128, 64], f32)
            t_b1 = wp.tile([64, 1], f32)
            t_w2 = wp.tile([128, 4], f32)
            t_nb2 = wp.tile([4, 1], f32)
            for tt, dd in [(t_cw, d_cw), (t_cb, d_cb), (t_w1, d_w1),
                           (t_b1, d_b1), (t_w2, d_w2), (t_nb2, d_nb2)]:
                nc.sync.dma_start(tt[:], dd[:])

            for blk in range(2):           # two row-blocks of 128 = 2 batches x 64 vars
                esc = blkp.tile([128, T], f32, tag="esc", name="esc")
                for vq in range(V // 4):   # 16 groups of 4 vars
                    # windows for the 2 var-pairs of this vq, both batches of blk
                    wnds = []
                    for vpl in range(2):
                        wnd = wndp.tile([6, 2 * L], f32, tag=f"wnd{vpl}", name=f"wnd{vpl}")
                        for sub in range(2):
                            v = 4 * vq + 2 * vpl + sub
                            for k in range(K):
                                row = 3 * sub + k
                                dst = wnd[row:row + 1, :].rearrange(
                                    "p (c t) -> p c t", c=2)[:, :, 0:T]
                                nc.sync.dma_start(
                                    dst, d_xs[2 * blk:2 * blk + 2, v, k:k + T]
                                )
                        wnds.append(wnd)
                    for bi in range(2):
                        r0 = bi * 64 + 4 * vq
                        eTmp = pool.tile([4, T], f32, tag="eTmp", name="eTmp")
                        for cs in range(4):
                            lo = cs * 512
                            n = CHS[cs]
                            h_c = hp.tile([128, 512], f32, tag="h_c", name="h_c")
                            for vpl in range(2):
                                vp = 2 * vq + vpl
                                fT = pool.tile([128, 512], f32, tag="fT", name="fT")
                                for ptl in range(4):
                                    w = PTW[cs][ptl]
                                    plo = bi * L + lo + ptl * 128
                                    flo = ptl * 128
                                    conv_ps = psp.tile([128, 128], f32, tag="conv_ps", name="conv_ps")
                                    nc.tensor.matmul(
                                        conv_ps[:w, :],
                                        wnds[vpl][:, plo:plo + w],
                                        t_cw[:, vp * 128:(vp + 1) * 128],
                                        start=True, stop=True,
                                    )
                                    ftc = pool.tile([128, 128], f32, tag="ftc", name="ftc")
                                    nc.scalar.copy(ftc[:w, :], conv_ps[:w, :])
                                    tr_ps = psp.tile([128, 128], f32, tag="tr_ps", name="tr_ps")
                                    nc.tensor.transpose(tr_ps[:, :w], ftc[:w, :], ident[:w, :w])
                                    nc.vector.tensor_scalar_add(
                                        fT[:, flo:flo + w], tr_ps[:, :w], t_cb[:, vp:vp + 1]
                                    )
                                pre1_ps = psp.tile([64, 512], f32, tag="pre1_ps", name="pre1_ps")
                                nc.tensor.matmul(pre1_ps[:, :n], t_w1[:], fT[:, :n], start=True, stop=True)
                                nc.scalar.activation(
                                    h_c[64 * vpl:64 * vpl + 64, :n], pre1_ps[:, :n],
                                    mybir.ActivationFunctionType.Tanh, bias=t_b1[:], scale=1.0,
                                )
                            pre2_ps = psp.tile([4, 512], f32, tag="pre2_ps", name="pre2_ps")
                            nc.tensor.matmul(pre2_ps[:, :n], t_w2[:], h_c[:, :n], start=True, stop=True)
                            nc.scalar.activation(
                                eTmp[:, lo:lo + n], pre2_ps[:, :n],
                                mybir.ActivationFunctionType.Exp, bias=t_nb2[:], scale=-1.0,
                            )
                        # ACT/DVE writes need 32-aligned partition bases; DMA does not.
                        nc.sync.dma_start(esc[r0:r0 + 4, :], eTmp[:])
                # finish sigmoid: scores = 1 / (esc + 1)
                nc.vector.tensor_scalar_add(esc[:], esc[:], 1.0)
                nc.vector.reciprocal(esc[:], esc[:])
                # extraction sort: 128 rounds of top-8
                idx_sb = blkp.tile([128, 1024], mybir.dt.uint32, tag="idx_sb", name="idx_sb")
                for r in range(128):
                    m8 = m8p.tile([128, 8], f32, tag="m8", name="m8")
                    nc.vector.max(out=m8[:], in_=esc[:])
                    nc.vector.max_index(out=idx_sb[:, 8 * r:8 * r + 8], in_max=m8[:], in_values=esc[:])
                    nc.vector.match_replace(out=esc[:], in_to_replace=m8[:], in_values=esc[:], imm_value=-1e30)
                nc.sync.dma_start(d_idx[blk * 128:(blk + 1) * 128, :], idx_sb[:, :TOPK])
    return nc


def _pack_weights(conv_w, conv_b, W1, b1, W2, b2):
    cw = np.zeros((6, (V // 2) * 128), dtype=np.float32)
    cb = np.zeros((128, V // 2), dtype=np.float32)
    for vp in range(V // 2):
        cw[0:3, vp * 128:vp * 128 + 64] = conv_w[2 * vp].T
        cw[3:6, vp * 128 + 64:vp * 128 + 128] = conv_w[2 * vp + 1].T
        cb[0:64, vp] = conv_b[2 * vp]
        cb[64:128, vp] = conv_b[2 * vp + 1]
    w1bd = np.zeros((128, 64), dtype=np.float32)
    w1bd[0:64, 0:32] = W1.T
    w1bd[64:128, 32:64] = W1.T
    w2bd = np.zeros((128, 4), dtype=np.float32)
    for j in range(4):
        w2bd[32 * j:32 * j + 32, j] = W2[0]
    b1p = np.concatenate([b1, b1]).reshape(64, 1).astype(np.float32)
    nb2 = np.full((4, 1), -float(b2[0]), dtype=np.float32)
    return cw, cb, w1bd, b1p, w2bd, nb2


def kernel(x, conv_w, conv_b, W1, b1, W2, b2):
    from concourse import bass_utils

    x = np.ascontiguousarray(x, dtype=np.float32)
    cw, cb, w1bd, b1p, w2bd, nb2 = _pack_weights(
        np.asarray(conv_w, np.float32), np.asarray(conv_b, np.float32),
        np.asarray(W1, np.float32), np.asarray(b1, np.float32),
        np.asarray(W2, np.float32), np.asarray(b2, np.float32))

    key = "nc"
    if key not in _cached:
        _cached[key] = _build_nc(float(nb2[0, 0]))
    nc = _cached[key]

    in_maps = []
    for c in range(NCORES):
        in_maps.append({
            "xs": np.ascontiguousarray(x[c * BLOC:(c + 1) * BLOC]),
            "cw": cw, "cb": cb, "w1": w1bd, "b1": b1p, "w2": w2bd, "nb2": nb2,
        })
    res = bass_utils.run_bass_kernel_spmd(nc, in_maps, core_ids=list(range(NCORES)))
    out = np.empty((B, V, TOPK), dtype=np.int32)
    for c in range(NCORES):
        idx = res.results[c]["idx"].astype(np.int64)  # [256, 1023]
        idx = idx.reshape(2, 2, V, TOPK)              # [blk, bi, v, k]
        for blk in range(2):
            for bi in range(2):
                out[c * BLOC + 2 * blk + bi] = idx[blk, bi].astype(np.int32)
    return out
